# revision 14
# baseline (speedup 1.0000x reference)
"""Low-rank layer y = (U^T V) @ x computed as y = U^T @ (V @ x).

Full problem: x [8192, 4096] f32, U/V [8, 8192] f32, y [8192, 4096] f32.
Sharding: batch (columns of x) split across 8 NeuronCores, 512 per core.

Design: NB=4 column blocks of CB=128 columns per core, all-bf16
matmuls (fp32 runs the PE at 1/4 rate; the input DMA casts f32 -> bf16
inline, which also halves SBUF-side DMA traffic), bf16 output stores
(host upcasts losslessly), software-pipelined: block b+1's phase-1 matmuls are emitted
inside block b's copy-paced phase-2 groups, so the PE stream never
head-of-line blocks and stores become ready almost as soon as the DMA
queue can take them. All loads SWDGE with inline f32->bf16 cast; stores
bf16 on HWDGE; last block stored in 0.5 MiB quarters to shorten the
tail. All 8 half-block x tiles and all 8 stage tiles stay resident.
Phase 2 computes four y chunks per standard matmul via block-diagonal
weights (U chunks stacked at 32-aligned partition bases, T replicated
on the diagonal of a zeroed [128, 512] rhs), cutting LDWEIGHTS 4x.
"""

import numpy as np

L = 8192
RANK = 8
BATCH = 4096
NCORES = 8
BS = BATCH // NCORES   # 512 batch columns per core
P = 128                # SBUF partitions
NCHUNK = L // P        # 64 row-chunks of 128
NB = 4                 # column blocks per core
CB = BS // NB          # 128 columns per block
HC = 32                # chunks per load-half / store-stage

_NC = None  # cached compiled Bass module


def _body(tc, nc, x, vt, u, y, mybir):
    from contextlib import ExitStack

    f32 = mybir.dt.float32
    bf16 = mybir.dt.bfloat16

    with ExitStack() as ctx:
        const = ctx.enter_context(tc.tile_pool(name="const", bufs=1))
        xpool = ctx.enter_context(tc.tile_pool(name="xb", bufs=2 * NB))
        warm = ctx.enter_context(tc.tile_pool(name="warm", bufs=1, space="PSUM"))
        tpsum = ctx.enter_context(tc.tile_pool(name="tpsum", bufs=2, space="PSUM"))
        tsb = ctx.enter_context(tc.tile_pool(name="tsb", bufs=2))
        ypsum = ctx.enter_context(tc.tile_pool(name="ypsum", bufs=4, space="PSUM"))
        ystage = ctx.enter_context(tc.tile_pool(name="ystage", bufs=2 * NB))

        # Tiny replicated operands, bf16 end to end.
        vt_sb = const.tile([P, NCHUNK * RANK], bf16)
        nc.sync.dma_start(vt_sb[:], vt[:])
        u128_sb = const.tile([P, (NCHUNK // 4) * P], bf16)
        nc.vector.memset(u128_sb[:], 0.0)
        for j in range(4):
            nc.sync.dma_start(u128_sb[32 * j:32 * j + RANK, :],
                              u[RANK * j:RANK * (j + 1), :])

        # Dummy matmuls absorbing the const-tensor DMA waits.
        warm1 = warm.tile([RANK, RANK], f32, tag="warm")
        nc.tensor.matmul(warm1[:], vt_sb[:, 0:RANK], vt_sb[:, 0:RANK],
                         start=True, stop=True)
        warm2 = warm.tile([P, RANK], f32, tag="warm")
        nc.tensor.matmul(warm2[:], u128_sb[0:RANK, 0:P], u128_sb[0:RANK, 0:RANK],
                         start=True, stop=True)

        # All loads issued up front: two 32-chunk halves per block, every
        # tile a distinct slot. SWDGE casts f32->bf16 inline.
        segs = {cb: [] for cb in range(NB)}
        for cb in range(NB):
            for h in range(2):
                xt = xpool.tile([P, HC * CB], bf16, tag="xt")
                off = (cb * NCHUNK + h * HC) * CB
                nc.gpsimd.dma_start(xt[:], x[:, off:off + HC * CB])
                segs[cb].append((xt, h * HC, HC))

        def p1_mm(cb, t_ps, n):
            for xt, lo, ln in segs[cb]:
                if lo <= n < lo + ln:
                    break
            nc.tensor.matmul(
                t_ps[:],
                vt_sb[:, n * RANK:(n + 1) * RANK],
                xt[:, (n - lo) * CB:(n - lo + 1) * CB],
                start=(n == 0),
                stop=(n == NCHUNK - 1),
                skip_group_check=True,
            )

        NG = NCHUNK // 4    # 16 phase-2 groups per block (4 chunks/bank)
        GS = NG // 2        # 8 groups per stage

        t_blk0 = tsb.tile([P, 4 * CB], bf16, tag="tc")
        t_blk1 = tsb.tile([P, 4 * CB], bf16, tag="tc")
        t_blks = [t_blk0, t_blk1]
        nc.gpsimd.memset(t_blk0[:], 0.0)
        nc.gpsimd.memset(t_blk1[:], 0.0)

        def p2_group(cb, t_blk, stage, g):
            # One matmul, four chunks: lhsT [32, 128] stacks the four
            # chunks' U-weights; rhs [32, 512] is T on the block diagonal
            # (zeros elsewhere), so out [128, 512] holds chunks 4g..4g+3
            # side by side. Cuts LDWEIGHTS count 4x vs per-chunk matmuls.
            y_ps = ypsum.tile([P, 4 * CB], f32, tag="yp")
            nc.tensor.matmul(
                y_ps[:],
                u128_sb[:, g * P:(g + 1) * P],
                t_blk[:],
                start=True,
                stop=True,
            )
            dst = stage[:, (g % GS) * 4 * CB:((g % GS) + 1) * 4 * CB]
            if g % 2 == 0:
                nc.scalar.copy(dst, y_ps[:])
            else:
                nc.vector.tensor_copy(dst, y_ps[:])

        # Block 0 phase 1 up front.
        t_ps = tpsum.tile([RANK, CB], f32, tag="t")
        for n in range(NCHUNK):
            p1_mm(0, t_ps, n)
        t_sb_cur = t_blks[0]
        for j in range(4):
            cp = nc.scalar.copy if j % 2 == 0 else nc.vector.tensor_copy
            cp(t_sb_cur[32 * j:32 * j + RANK, j * CB:(j + 1) * CB], t_ps[:])

        for cb in range(NB):
            last = (cb == NB - 1)
            if not last:
                t_ps_next = tpsum.tile([RANK, CB], f32, tag="t")
            stage = None
            for g in range(NG):
                if g % GS == 0:
                    stage = ystage.tile([P, HC * CB], bf16, tag="ys")
                p2_group(cb, t_sb_cur, stage, g)
                if not last:
                    for k in range(4):
                        p1_mm(cb + 1, t_ps_next, g * 4 + k)
                if last and g % (GS // 2) == (GS // 2) - 1:
                    # tail block: 16-chunk (0.5 MiB) quarter stores
                    e = g // (GS // 2)
                    off = (cb * NCHUNK + e * (HC // 2)) * CB
                    part = (e % 2) * (HC // 2) * CB
                    nc.sync.dma_start(
                        y[:, off:off + (HC // 2) * CB],
                        stage[:, part:part + (HC // 2) * CB])
                elif not last and g % GS == GS - 1:
                    h = g // GS
                    off = (cb * NCHUNK + h * HC) * CB
                    nc.sync.dma_start(y[:, off:off + HC * CB], stage[:])
            if not last:
                t_sb_cur = t_blks[(cb + 1) % 2]
                for j in range(4):
                    cp = nc.scalar.copy if j % 2 == 0 else nc.vector.tensor_copy
                    cp(t_sb_cur[32 * j:32 * j + RANK, j * CB:(j + 1) * CB],
                       t_ps_next[:])


def build_bass():
    import concourse.mybir as mybir
    import concourse.tile as tile
    from concourse import bacc

    nc = bacc.Bacc("TRN2", target_bir_lowering=False, debug=False)
    x = nc.dram_tensor("x", [P, NB * NCHUNK * CB], mybir.dt.float32,
                       kind="ExternalInput").ap()
    vt = nc.dram_tensor("vt", [P, NCHUNK * RANK], mybir.dt.bfloat16,
                        kind="ExternalInput").ap()
    u = nc.dram_tensor("u", [4 * RANK, (NCHUNK // 4) * P], mybir.dt.bfloat16,
                       kind="ExternalInput").ap()
    y = nc.dram_tensor("y", [P, NB * NCHUNK * CB], mybir.dt.bfloat16,
                       kind="ExternalOutput").ap()

    with tile.TileContext(nc) as tc:
        _body(tc, nc, x, vt, u, y, mybir)
    nc.compile()
    return nc


def _get_nc():
    global _NC
    if _NC is None:
        _NC = build_bass()
    return _NC


def make_in_maps(inputs, U, V):
    import ml_dtypes

    x = np.asarray(inputs, dtype=np.float32)
    U = np.asarray(U, dtype=np.float32)
    V = np.asarray(V, dtype=np.float32)
    ub = np.ascontiguousarray(
        U.reshape(RANK, NCHUNK // 4, 4, P).transpose(2, 0, 1, 3)
        .reshape(4 * RANK, -1)).astype(ml_dtypes.bfloat16)
    vt = np.ascontiguousarray(
        V.reshape(RANK, NCHUNK, P).transpose(2, 1, 0).reshape(P, NCHUNK * RANK)
    ).astype(ml_dtypes.bfloat16)
    in_maps = []
    for c in range(NCORES):
        xs = x[:, c * BS:(c + 1) * BS]
        xb = np.ascontiguousarray(
            xs.reshape(NCHUNK, P, NB, CB).transpose(1, 2, 0, 3).reshape(P, -1)
        )
        in_maps.append({"x": xb, "vt": vt, "u": ub})
    return in_maps


def _unblock_y(yb):
    return np.ascontiguousarray(
        np.asarray(yb).reshape(P, NB, NCHUNK, CB)
        .transpose(2, 0, 1, 3).reshape(L, BS)
    ).astype(np.float32)


def kernel(inputs, U, V):
    from concourse import bass_utils

    nc = _get_nc()
    in_maps = make_in_maps(inputs, U, V)
    res = bass_utils.run_bass_kernel_spmd(nc, in_maps, core_ids=list(range(NCORES)))
    return np.concatenate(
        [_unblock_y(res.results[c]["y"]) for c in range(NCORES)], axis=1)
